# revision 7
# baseline (speedup 1.0000x reference)
"""Trainium2 Bass kernel for MinimalLightningIndexer.

out[b,t,s] = relu((x@Wq)[b,t] . (x@Wk)[b,s]) * (x@Ww)[b,t]

Sharding: token-parallel across all batches. Core c owns tokens
[c*512,(c+1)*512) of every batch (2048 query tokens per core) and
produces the [2048, 4096] score rows for them. Each core loads only its
own x slice (8 MB bf16), projects q/w/k in one fused pass (stationary
[Wq|0|Wk|0|Ww], 65 wide, 32-aligned groups), then an 8-way AllGather
shares the tiny k
projections (32 KB per rank per group) so no core ever loads peer x.
Every core consumes the whole AllGather result, keeping the SPMD
program identical across cores.

Per-core device program:
  - 4 x.T chunk loads [2048d x 512tok] (2 MB DMAs)
  - PE: psqwk[33,512] per chunk (16 accumulating matmuls, K=128)
  - k rows staged to DRAM; two AllGathers (batches 0-1, 2-3) overlap
    the remaining input/projection work
  - gathered k read into two 32-aligned partition bands; scores run as
    row-tiled matmul pairs (two concurrent K=16 matmuls via PE row
    groups) into [128,1024] PSUM tiles
  - epilogue relu*gate: alternating fused VectorE tensor_scalar
    (max 0, mult w) and ScalarE relu + VectorE mul, into [128,4096]
    bf16 output tiles, 1 MB output DMAs
"""

import sys

if "/opt/trn_rl_repo" not in sys.path:
    sys.path.insert(0, "/opt/trn_rl_repo")

import numpy as np

import concourse.bacc as bacc
import concourse.bass as bass
import concourse.mybir as mybir
import concourse.tile as tile
from concourse.bass_utils import run_bass_kernel_spmd

B, S, D = 4, 4096, 2048
IDX = 16
# fused projection stationary: q @ cols 0-15, k @ cols 32-47, w @ col 64
# (32-aligned groups so each PSUM->SBUF engine copy starts 32-aligned)
QWK = 65
N_CORES = 8
TOK = 2048            # own query tokens per core (B x 512)
DC = D // 128         # 16 d-chunks of 128
CH = 4                # token chunks of 512 (chunk j = batch j's tokens)
TT = TOK // 128       # 16 t-tiles

_CACHE = {}


def _build_nc():
    if "nc" in _CACHE:
        return _CACHE["nc"]
    f32 = mybir.dt.float32
    bf16 = mybir.dt.bfloat16
    nc = bacc.Bacc("TRN2", target_bir_lowering=False, debug=False,
                   num_devices=N_CORES)
    xt = nc.dram_tensor("xt", [D, TOK], bf16, kind="ExternalInput").ap()
    wqwk = nc.dram_tensor("wqwk", [D, QWK], bf16, kind="ExternalInput").ap()
    o = nc.dram_tensor("o", [TOK, S], bf16, kind="ExternalOutput").ap()

    with tile.TileContext(nc) as tc:
        with (
            tc.tile_pool(name="const", bufs=1) as cpool,
            tc.tile_pool(name="slab", bufs=2) as slab_pool,
            tc.tile_pool(name="osb", bufs=3) as out_pool,
            tc.tile_pool(name="pqwk", bufs=1, space="PSUM") as pq_pool,
            tc.tile_pool(name="ps", bufs=3, space="PSUM") as ps_pool,
            tc.tile_pool(name="dram", bufs=1, space="DRAM") as dram,
        ):
            # --- persistent tensors ---
            wqwk_sb = cpool.tile([128, DC * QWK], bf16, tag="wqwk_sb")
            nc.sync.dma_start(
                out=wqwk_sb[:],
                in_=wqwk.rearrange("(kd p) i -> p kd i", p=128),
            )
            # q stationary, two row bands (partitions 0-15 and 32-47)
            qrep = cpool.tile([48, TOK], bf16, tag="qrep")
            # gate row and its [128, TT] transpose
            wrow = cpool.tile([1, TOK], f32, tag="wrow")
            w_col = cpool.tile([128, TT], f32, tag="w_col")
            # k staging for AllGather input
            kst_sb = cpool.tile([16, TOK], bf16, tag="kst_sb")
            # gathered k, two row bands; cols = b*4096 + s
            kt_rep = cpool.tile([48, B * S], bf16, tag="kt_rep")

            kstage = [dram.tile([16, 1024], bf16, name=f"kstage{g}")
                      for g in range(2)]
            gout = [dram.tile([128, 1024], bf16, name=f"gout{g}")
                    for g in range(2)]

            # --- projections per 512-token chunk (chunk j = batch j) ---
            for j in range(CH):
                slab = slab_pool.tile([128, DC * 512], bf16, tag="slab")
                nc.sync.dma_start(
                    out=slab[:],
                    in_=xt[:, j * 512:(j + 1) * 512].rearrange(
                        "(kd p) s -> p kd s", p=128),
                )
                slab_v = slab[:].rearrange("p (kd t) -> p kd t", kd=DC)

                psq = pq_pool.tile([QWK, 512], f32, tag="psq")
                for kd in range(DC):
                    nc.tensor.matmul(
                        psq[:],
                        wqwk_sb[:, kd * QWK:(kd + 1) * QWK],
                        slab_v[:, kd, :],
                        start=(kd == 0), stop=(kd == DC - 1),
                    )
                cols = slice(j * 512, (j + 1) * 512)
                # q -> band 0 of stationary, bf16 (VectorE: keep ScalarE
                # on a single activation table-set, Relu only)
                nc.vector.tensor_copy(qrep[0:16, cols], psq[0:16, :])
                # w -> f32 row
                nc.vector.tensor_copy(wrow[0:1, cols], psq[64:65, :])
                # k -> bf16 staging
                nc.vector.tensor_copy(kst_sb[:, cols], psq[32:48, :])
                # stage k chunk to DRAM for the collective
                g, half = j // 2, j % 2
                nc.sync.dma_start(
                    out=kstage[g][:, half * 512:(half + 1) * 512],
                    in_=kst_sb[:, cols],
                )
                # replicate q into row band 1 (partitions 32-47)
                nc.sync.dma_start(out=qrep[32:48, cols], in_=qrep[0:16, cols])
                # transpose gate pieces: w_col[:, ti] for this chunk's t-tiles
                for t in range(4):
                    ti = j * 4 + t
                    nc.sync.dma_start(
                        out=w_col[:, ti:ti + 1],
                        in_=wrow[0:1, ti * 128:(ti + 1) * 128],
                    )
                # fire the group's AllGather once both its chunks are staged
                if half == 1:
                    nc.gpsimd.collective_compute(
                        "AllGather",
                        mybir.AluOpType.bypass,
                        replica_groups=[list(range(N_CORES))],
                        ins=[kstage[g].opt()],
                        outs=[gout[g].opt()],
                    )
                    for b2 in range(2):
                        b = g * 2 + b2
                        nc.sync.dma_start(
                            out=kt_rep[0:16, b * S:(b + 1) * S].rearrange(
                                "i (r u) -> i r u", r=N_CORES),
                            in_=gout[g][:, b2 * 512:(b2 + 1) * 512].rearrange(
                                "(r i) u -> i r u", i=16),
                        )
                    nc.sync.dma_start(
                        out=kt_rep[32:48, g * 2 * S:(g + 1) * 2 * S],
                        in_=kt_rep[0:16, g * 2 * S:(g + 1) * 2 * S],
                    )

            # --- scores ---
            for b in range(B):
                for tt in range(4):
                    ti = b * 4 + tt
                    osb = out_pool.tile([128, S], bf16, tag="osb")
                    for pg in range(4):
                        pss = ps_pool.tile([128, 1024], f32, tag="pss")
                        for h in range(2):
                            sc = 2 * pg + h
                            nc.tensor.matmul(
                                pss[:, h * 512:(h + 1) * 512],
                                qrep[32 * h:32 * h + 16,
                                     ti * 128:(ti + 1) * 128],
                                kt_rep[32 * h:32 * h + 16,
                                       b * S + sc * 512:b * S + (sc + 1) * 512],
                                start=True, stop=True,
                            )
                        oslice = osb[:, 2 * pg * 512:(2 * pg + 2) * 512]
                        if pg % 2 == 0:
                            nc.vector.tensor_scalar(
                                oslice, pss[:],
                                scalar1=0.0,
                                scalar2=w_col[:, ti:ti + 1],
                                op0=mybir.AluOpType.max,
                                op1=mybir.AluOpType.mult,
                            )
                        else:
                            nc.scalar.activation(
                                oslice, pss[:],
                                mybir.ActivationFunctionType.Relu,
                            )
                            nc.vector.tensor_scalar_mul(
                                out=oslice, in0=oslice,
                                scalar1=w_col[:, ti:ti + 1],
                            )
                    nc.sync.dma_start(
                        out=o[ti * 128:(ti + 1) * 128, :],
                        in_=osb[:],
                    )
    nc.compile()
    _CACHE["nc"] = nc
    return nc


def _make_in_maps(x, Wq, Wk, Ww):
    import ml_dtypes
    bf = ml_dtypes.bfloat16
    wqwk = np.zeros((D, QWK), dtype=np.float32)
    wqwk[:, 0:16] = Wq
    wqwk[:, 32:48] = Wk
    wqwk[:, 64:65] = Ww
    wqwk = np.ascontiguousarray(wqwk).astype(bf)
    xbf = x.astype(bf)
    in_maps = []
    for c in range(N_CORES):
        own = xbf[:, c * 512:(c + 1) * 512, :].reshape(TOK, D)
        xt = np.ascontiguousarray(own.T)
        in_maps.append({"xt": xt, "wqwk": wqwk})
    return in_maps


def _assemble(results):
    out = np.empty((B, S, S), dtype=np.float32)
    for c in range(N_CORES):
        oc = np.asarray(results[c]["o"], dtype=np.float32)
        out[:, c * 512:(c + 1) * 512, :] = oc.reshape(B, 512, S)
    return out


def kernel(x, Wq, Wk, Ww, _trace_kwargs=None):
    nc = _build_nc()
    in_maps = _make_in_maps(np.asarray(x, dtype=np.float32),
                            np.asarray(Wq, dtype=np.float32),
                            np.asarray(Wk, dtype=np.float32),
                            np.asarray(Ww, dtype=np.float32))
    kw = _trace_kwargs or {}
    res = run_bass_kernel_spmd(nc, in_maps, list(range(N_CORES)), **kw)
    out = _assemble(res.results)
    if _trace_kwargs is not None:
        return out, res
    return out
